# revision 19
# baseline (speedup 1.0000x reference)
"""Single attention head (B=8, S=2048, D_IN=1024, D_OUT=64) on 8 TRN2 NeuronCores.

Strategy: pure data-parallel over batch -- core b computes batch element b's
full attention head. No collectives.

v6 -- three-engine softmax + untangled startup:
  - Scores are tiny (|x| <~ 0.33), so exp(x) ~= ((1+x)^2 + 1)/2 to 6e-3 worst
    case / 3e-5 rms.  Pairs 0,1 of every q-chunk compute their weights as
    t' = (1+x)/sqrt(2) on the DVE (fp16), squared on the otherwise-idle
    GpSimd (tensor_mul, bf16 out); pairs 2-7 stay ScalarE exps.  The
    dropped "+0.5" per quad weight is a q-independent rank-1 correction
    c = 0.5 * [V|keep]^T keep over quad key chunks 0..3, computed by 4
    one-column PE matmuls and folded into finalize's psum->sbuf drain
    (tensor_scalar_add).  ScalarE exp wall 36.7us -> ~27us.
  - The ScalarE is also a DMA-issue engine, and issue instructions that
    wait on ring credit block the exp stream behind them (measured: first
    exp pushed to 16.4us).  So the scalar HWDGE queue gets only the first
    few never-blocking DMAs (f8_0a, f8_1, cold, sb halves of sj0/1); the
    bulk streams on sync.  Host layouts are partition-outermost so any
    [sj/c-range] slice is one contiguous-per-partition descriptor, keeping
    the total at 13 DMAs.
  - The exp-table preload reads a memset tile (no DMA dep) so the ~2.7us
    table DMA runs at ~6.5us, before the issue instructions queue up.
  - Warmup matmuls run on the same memset tile with no DMA dependency:
    HAM reaches 8/8 while the first input chunks are still in flight.
  - K/Q projections alternate their psum tiles between the pk and ctx
    pools so proj(s+1) never waits on proj(s-1)'s DVE bias-adds (the ctx
    pool is idle until the first ctx pop, ~10us later).
  - qc0's ScalarE pairs interleave with the projections; its two quad
    pairs come last so the proj ladder never waits on a 1.2us DVE op.
  - finalize runs in fp16 (PE transpose w/ fp16 identity): half the f32
    transpose cycles, 2x DVE rate on the drain.
  - ctx pops are gated on the v-units already emitted (program order =
    dependency order) but pop out of FIFO order so a blocked qc0 tail
    doesn't stall qc1 ctx work.
Masking: masked keys' V rows and the keep-column (denominator) are zeroed
on-chip; scores are never masked (exact).
"""

import numpy as np
import ml_dtypes

import concourse.bass as bass  # noqa: F401  (bass types used via tile/bacc)
import concourse.mybir as mybir
import concourse.tile as tile
from concourse import bacc
from concourse.bass_utils import run_bass_kernel_spmd

B, S, D, F = 8, 2048, 1024, 64
NCORES = 8
BF = mybir.dt.bfloat16
F16 = mybir.dt.float16
F8 = mybir.dt.float8e4
F32 = mybir.dt.float32
# reference scales by sqrt(S); q and k each carry x32 from the fp8 weight scaling
SCALE = 1.0 / (1024.0 * float(np.sqrt(np.float32(S))))
SC = 512  # matmul moving free-dim
NSJ = S // SC  # 4 column chunks of the projection loop
KCH = S // 128  # 16 key chunks
DCH = D // 128  # 8 bf16 contraction chunks
DR = D // 256  # 4 fp8 DoubleRow contraction chunks
N_QUAD = 2  # pairs 0..N_QUAD-1 of every qc use the DVE/GpSimd quadratic
HOT_B = 2124  # wkq 1024 | wqk 1024 | misc 76
COLD_B = 1410  # wv 1024 | identb 256 | identh 130


def _emit(nc):
    # partition-outermost host layouts: any [sj, c-range] slice is a single
    # contiguous-per-partition DMA descriptor
    seqf8_d = nc.declare_dram_parameter("seqf8", [128, NSJ, DR, 2, SC], F8, isOutput=False)
    seqb_d = nc.declare_dram_parameter("seqb", [128, NSJ, DCH, SC], BF, isOutput=False)
    ch_d = nc.declare_dram_parameter("ch", [128, HOT_B], mybir.dt.uint8, isOutput=False)
    cc_d = nc.declare_dram_parameter("cc", [128, COLD_B], mybir.dt.uint8, isOutput=False)
    out_d = nc.declare_dram_parameter("out", [S, F], F32, isOutput=True)

    with tile.TileContext(nc) as tc:
        _body(nc, tc, seqf8_d, seqb_d, ch_d, cc_d, out_d)
    nc.compile()


def _body(nc, tc, seqf8_d, seqb_d, ch_d, cc_d, out_d):
    from contextlib import ExitStack

    with ExitStack() as ctx:
        const = ctx.enter_context(tc.tile_pool(name="const", bufs=1))
        big = ctx.enter_context(tc.tile_pool(name="big", bufs=1))
        sbw = ctx.enter_context(tc.tile_pool(name="sbw", bufs=1))
        ps = ctx.enter_context(tc.tile_pool(name="ps", space="PSUM", bufs=1))

        # ---- HAM warmup + exp-table preload on a memset tile: no DMA deps ----
        warm_sb = const.tile([128, SC], F8, name="warm_sb")
        nc.gpsimd.memset(warm_sb[:], 0)
        dummy_sb = const.tile([1, 1], F32, name="dummy_sb")
        nc.scalar.activation(
            out=dummy_sb[:],
            in_=warm_sb[0:1, 0:4].bitcast(F32),
            func=mybir.ActivationFunctionType.Exp,
            scale=1.0,
        )
        # ~4.5us of junk matmuls: long enough to pull HAM through the SHORT
        # window to 8/8 while the first input DMAs (~5us latency) land
        for i in range(13):
            ps_warm = ps.tile([128, SC], F32, tag="pk", bufs=2, name=f"ps_warm{i}")
            nc.tensor.matmul(
                ps_warm[:], warm_sb[:, 0:128], warm_sb[:], start=True, stop=True
            )

        # ---- input DMAs.  scalar queue: only early, never-blocking issues
        # (a blocked issue instruction stalls the exp stream behind it in
        # ScalarE's FIFO); sync queue: the bulk, ordered by first use ----
        hot_sb = const.tile([128, HOT_B], mybir.dt.uint8, name="hot_sb")
        cold_sb = const.tile([128, COLD_B], mybir.dt.uint8, name="cold_sb")
        f8_00 = big.tile([128, 1, 2, SC], F8, name="f8_00")
        f8_01 = big.tile([128, 1, 2, SC], F8, name="f8_01")
        f8_0b = big.tile([128, 2, 2, SC], F8, name="f8_0b")
        seqf8 = [None] + [
            big.tile([128, DR, 2, SC], F8, name=f"seqf8_{j}") for j in range(1, NSJ)
        ]
        # sj0/sj1 in half-chunks (feed B-phase V units early); sj2/sj3 whole
        seqbh = [
            big.tile([128, DCH // 2, SC], BF, name=f"seqb_{j}_{h}")
            for j in range(2)
            for h in range(2)
        ]
        seqbf = [big.tile([128, DCH, SC], BF, name=f"seqb_{j}") for j in (2, 3)]

        nc.scalar.dma_start(out=f8_00[:], in_=seqf8_d[:, 0, 0:1, :, :])
        nc.sync.dma_start(out=hot_sb[:], in_=ch_d.ap())
        nc.scalar.dma_start(out=f8_01[:], in_=seqf8_d[:, 0, 1:2, :, :])
        nc.sync.dma_start(out=f8_0b[:], in_=seqf8_d[:, 0, 2:4, :, :])
        nc.scalar.dma_start(out=seqf8[1][:], in_=seqf8_d[:, 1])
        nc.sync.dma_start(out=seqf8[2][:], in_=seqf8_d[:, 2])
        nc.sync.dma_start(out=seqf8[3][:], in_=seqf8_d[:, 3])
        nc.scalar.dma_start(out=cold_sb[:], in_=cc_d.ap())
        nc.scalar.dma_start(out=seqbh[0][:], in_=seqb_d[:, 0, 0:4, :])
        nc.sync.dma_start(out=seqbh[1][:], in_=seqb_d[:, 0, 4:8, :])
        nc.scalar.dma_start(out=seqbh[2][:], in_=seqb_d[:, 1, 0:4, :])
        nc.sync.dma_start(out=seqbh[3][:], in_=seqb_d[:, 1, 4:8, :])
        nc.sync.dma_start(out=seqbf[0][:], in_=seqb_d[:, 2])
        nc.sync.dma_start(out=seqbf[1][:], in_=seqb_d[:, 3])

        def seqb_half(u):
            sj, h = divmod(u, 2)
            if sj < 2:
                return seqbh[u][:, :, :]
            return seqbf[sj - 2][:, 4 * h : 4 * h + 4, :]

        wkq_sb = hot_sb[:, 0:1024].bitcast(F8).rearrange(
            "p (c i f) -> p c i f", c=DR, i=2
        )
        wqk_sb = hot_sb[:, 1024:2048].bitcast(F8).rearrange(
            "p (c i f) -> p c i f", c=DR, i=2
        )
        misc_sb = hot_sb[:, 2048:2124].bitcast(F32)
        wv_sb = cold_sb[:, 0:1024].bitcast(BF).rearrange("p (c f) -> p c f", c=DCH)
        identb_sb = cold_sb[:, 1024:1280].bitcast(BF)
        identh_sb = cold_sb[:, 1280:1410].bitcast(F16)

        # 0.5-column for the quad-correction matmuls
        halfones = const.tile([128, 1], BF, name="halfones")
        nc.gpsimd.memset(halfones[:], 0.5)

        # kqT: k on rows 0:64 (pair A lhsT), q on rows 64:128 (pair B rhs)
        # kq2T (swapped): q on rows 0:64 (pair A rhs), k on rows 64:128
        # (pair B lhsT).  Streamed tensors are per-sj/per-qc TILES so
        # tile-granular deps don't gate early readers on late writers.
        kqT = [big.tile([128, SC], BF, name=f"kqT_{j}") for j in range(NSJ)]
        kq2T = [big.tile([128, SC], BF, name=f"kq2T_{j}") for j in range(NSJ)]
        vT = [big.tile([F, SC], BF, name=f"vT_{j}") for j in range(NSJ)]
        v_sbs = [big.tile([128, 4, F + 1], BF, name=f"v_sb{j}") for j in range(NSJ)]
        out_sbs = [big.tile([128, 4, F], F32, name=f"out_sb{q}") for q in range(4)]
        c_sb = sbw.tile([F + 1, 1], F32, name="c_sb")
        out_r = out_d.ap().rearrange("(c p) f -> p c f", p=128)

        bkq_ap = misc_sb[:, 0:1]  # stacked 32*[bk; bq]
        bqk_ap = misc_sb[:, 1:2]  # stacked 32*[bq; bk]
        bv_ap = misc_sb[0:F, 2:3]
        mask01 = misc_sb[:, 3:]  # [128, 16] 1.0 = keep, 0.0 = masked out

        # ones-column of v := keep-mask (masked keys contribute 0 to the sums)
        for j in range(NSJ):
            nc.gpsimd.tensor_copy(v_sbs[j][:, :, F], mask01[:, 4 * j : 4 * j + 4])

        ctx_tiles = {}
        pending_ctx = []  # deferred ctx matmuls -- popped as PE filler
        units_done = 0  # v-units emitted so far; gates which entries may pop
        pairs_emitted = 0  # age counter for pop gating

        def emit_ctx(qc, p, wA, wB):
            # start/stop follow EMISSION order (pops may run out of p-order):
            # the first emitted matmul clears the bank, the 16th closes it
            ctx_ps = ctx_tiles[qc]
            ka, kb = 2 * p, 2 * p + 1
            nc.tensor.matmul(
                ctx_ps[:],
                v_sbs[ka // 4][:, ka % 4, :],
                wA,
                start=(done_ctx[qc] == 0),
                stop=False,
            )
            nc.tensor.matmul(
                ctx_ps[:],
                v_sbs[kb // 4][:, kb % 4, :],
                wB,
                start=False,
                stop=(done_ctx[qc] == KCH // 2 - 1),
            )

        done_ctx = {qc: 0 for qc in range(4)}

        def pop_ctx(n, force=False):
            # pop up to n READY entries: the v-units covering key chunks
            # 2p, 2p+1 must already be emitted (program order = dep order),
            # and the entry must be old enough that its weights have likely
            # been produced (quads ride the slower DVE+GpSimd pipe).
            popped = 0
            i = 0
            while popped < n and i < len(pending_ctx):
                qc, p, wA, wB, born, age = pending_ctx[i]
                chunks_ready = 4 * (units_done // 2)  # unit h=1 completes its sj
                if 2 * p + 1 >= chunks_ready or (
                    not force and pairs_emitted - born < age
                ):
                    i += 1
                    continue
                if qc not in ctx_tiles:
                    ctx_tiles[qc] = ps.tile(
                        [F + 1, SC], F32, tag="ctx", bufs=2, name=f"ctx_ps{qc}"
                    )
                pending_ctx.pop(i)
                emit_ctx(qc, p, wA, wB)
                done_ctx[qc] += 1
                popped += 1
                if done_ctx[qc] == KCH // 2:
                    finalize(qc)

        def pair_block(qc, p):
            # scores for key chunks (2p, 2p+1) x q-chunk qc.  Softmax weights
            # via ScalarE exp, or the DVE+GpSimd quadratic for quad pairs.
            nonlocal pairs_emitted
            ka, kb = 2 * p, 2 * p + 1
            ps_pair = ps.tile(
                [128, 2 * SC], F32, tag="pair", bufs=2, name=f"ps_pair_{qc}_{p}"
            )
            # chunk A on array rows 0:64, chunk B on rows 64:128 --
            # disjoint row groups run concurrently on the PE
            nc.tensor.matmul(
                ps_pair[:, 0:SC],
                kqT[ka // 4][0:F, (ka % 4) * 128 : (ka % 4 + 1) * 128],
                kq2T[qc][0:F, :],
                start=True,
                stop=True,
            )
            nc.tensor.matmul(
                ps_pair[:, SC : 2 * SC],
                kq2T[kb // 4][64:128, (kb % 4) * 128 : (kb % 4 + 1) * 128],
                kqT[qc][64:128, :],
                start=True,
                stop=True,
            )
            if p < N_QUAD:
                # t' = (1+x)/sqrt(2); gpsimd squares: w = (1+x)^2/2.  Half
                # tiles so a ctx matmul only waits its own half's square.
                # The +0.5 lands later via the c_sb correction column.
                rh = float(np.sqrt(0.5))
                sq = []
                for h in range(2):
                    tq = sbw.tile(
                        [128, SC], F16, tag="tq", bufs=8, name=f"tq_{qc}_{p}_{h}"
                    )
                    nc.vector.tensor_scalar(
                        tq[:], ps_pair[:, h * SC : (h + 1) * SC], SCALE * rh, rh,
                        mybir.AluOpType.mult, mybir.AluOpType.add,
                    )
                    s = sbw.tile(
                        [128, SC], BF, tag="sq", bufs=8, name=f"sq_{qc}_{p}_{h}"
                    )
                    nc.gpsimd.tensor_mul(s[:], tq[:], tq[:])
                    sq.append(s)
                pending_ctx.append((qc, p, sq[0][:], sq[1][:], pairs_emitted, 3))
            elif qc == 3 and p == 7:
                # the very last weights: two half-exps so the final ctx
                # matmuls start ~0.7us earlier (shorter serial tail)
                eA = sbw.tile([128, SC], BF, tag="sq", bufs=8, name="expq_3_7a")
                eB = sbw.tile([128, SC], BF, tag="sq", bufs=8, name="expq_3_7b")
                for h, e in enumerate((eA, eB)):
                    nc.scalar.activation(
                        out=e[:],
                        in_=ps_pair[:, h * SC : (h + 1) * SC],
                        func=mybir.ActivationFunctionType.Exp,
                        scale=SCALE,
                    )
                pending_ctx.append((qc, p, eA[:], eB[:], pairs_emitted, 1))
            else:
                expq = sbw.tile(
                    [128, 2 * SC], BF, tag="expq", bufs=12, name=f"expq_{qc}_{p}"
                )
                nc.scalar.activation(
                    out=expq[:],
                    in_=ps_pair[:],
                    func=mybir.ActivationFunctionType.Exp,
                    scale=SCALE,
                )
                pending_ctx.append(
                    (qc, p, expq[:, 0:SC], expq[:, SC : 2 * SC], pairs_emitted, 1)
                )
            pairs_emitted += 1

        # ---- V-projection filler units (PE work between exp-paced blocks) ----
        vps = {}

        def v_unit(u):
            nonlocal units_done
            sj, h = divmod(u, 2)
            if h == 0:
                vps[sj] = ps.tile([F, SC], F32, tag="pk", bufs=2, name=f"ps_v{sj}")
            for c in range(4 * h, 4 * h + 4):
                nc.tensor.matmul(
                    vps[sj][:],
                    wv_sb[:, c, :],
                    seqb_half(u)[:, c - 4 * h, :],
                    start=(c == 0),
                    stop=(c == DCH - 1),
                )
            if h == 1:
                nc.vector.tensor_scalar_add(vT[sj][:], vps[sj][:], bv_ap)
                # transpose into natural [k, f] layout on the PE
                for i in range(4):
                    t = 4 * sj + i
                    vtp = ps.tile([128, F], BF, tag="pk", bufs=2, name=f"vtp{t}")
                    nc.tensor.transpose(
                        vtp[:],
                        vT[sj][:, i * 128 : (i + 1) * 128],
                        identb_sb[0:F, 0:F],
                    )
                    nc.vector.tensor_scalar_mul(
                        v_sbs[sj][:, i, 0:F], vtp[:], mask01[:, t : t + 1]
                    )
            units_done = u + 1

        def emit_correction():
            # c = 0.5 * [V | keep]^T * keep over quad key chunks 0..2*N_QUAD-1
            # (pairs 0..N_QUAD-1 of every qc); q-independent, so one column.
            corr_ps = ps.tile([F + 1, 1], F32, tag="pk", bufs=2, name="corr_ps")
            nq = 2 * N_QUAD
            for t in range(nq):
                nc.tensor.matmul(
                    corr_ps[:],
                    v_sbs[t // 4][:, t % 4, :],
                    halfones[:],
                    start=(t == 0),
                    stop=(t == nq - 1),
                )
            nc.vector.tensor_copy(c_sb[:], corr_ps[:])

        def finalize(qc):
            # drain ctx (+ quad correction), reciprocal the denominator row,
            # PE-transpose back to [q, 65] in fp16, scale rows, store
            ctx_ps = ctx_tiles.pop(qc)
            ctxTq = sbw.tile([F + 1, SC], F16, tag="ctxTq", bufs=2, name=f"ctxTq{qc}")
            nc.vector.tensor_scalar_add(ctxTq[:], ctx_ps[:], c_sb[0 : F + 1, 0:1])
            for i in range(SC // 128):
                t = qc * 4 + i
                ctp = ps.tile([128, F + 1], F16, tag="pk", bufs=2, name=f"ctp{t}")
                nc.tensor.transpose(
                    ctp[:],
                    ctxTq[:, i * 128 : (i + 1) * 128],
                    identh_sb[0 : F + 1, 0 : F + 1],
                )
                rec = sbw.tile([128, 1], F32, tag="rec", bufs=4, name=f"rec{t}")
                nc.vector.reciprocal(rec[:], ctp[:, F : F + 1])
                nc.vector.tensor_scalar_mul(
                    out_sbs[qc][:, i, :], ctp[:, 0:F], rec[:]
                )
                if qc == 3 and i == 1:
                    # half-way store so the final DMA only covers 2 blocks
                    nc.sync.dma_start(
                        out=out_r[:, 12:14, :], in_=out_sbs[3][:, 0:2, :]
                    )
            if qc == 3:
                nc.sync.dma_start(
                    out=out_r[:, 14:16, :], in_=out_sbs[3][:, 2:4, :]
                )
            else:
                nc.sync.dma_start(
                    out=out_r[:, qc * 4 : (qc + 1) * 4, :],
                    in_=out_sbs[qc][:],
                )

        # ---- Phase A: K/Q projections, psum alternating between the pk and
        # ctx pools (ctx is idle until the first pop) so proj(s+1) never
        # waits on proj(s-1)'s bias-adds ----
        def proj(sj):
            tag = "pk" if sj % 2 == 0 else "ctx"
            ps_kq = ps.tile([128, SC], F32, tag=tag, bufs=2, name=f"ps_kq{sj}")
            ps_kq2 = ps.tile([128, SC], F32, tag=tag, bufs=2, name=f"ps_kq2_{sj}")
            for c in range(DR):
                if sj == 0:
                    if c < 2:
                        rhs = (f8_00 if c == 0 else f8_01)[:, 0, :, :]
                    else:
                        rhs = f8_0b[:, c - 2, :, :]
                else:
                    rhs = seqf8[sj][:, c, :, :]
                st = dict(start=(c == 0), stop=(c == DR - 1))
                nc.tensor.matmul(
                    ps_kq[:], wkq_sb[:, c, :, :], rhs,
                    perf_mode=mybir.MatmulPerfMode.DoubleRow, **st
                )
                nc.tensor.matmul(
                    ps_kq2[:], wqk_sb[:, c, :, :], rhs,
                    perf_mode=mybir.MatmulPerfMode.DoubleRow, **st
                )
            if sj < 2:
                # ScalarE drains the early projections (it is idle until the
                # first exp; keeps the DVE off the phase-A critical path)
                nc.scalar.activation(
                    out=kqT[sj][:], in_=ps_kq[:],
                    func=mybir.ActivationFunctionType.Identity,
                    bias=bkq_ap, scale=1.0,
                )
                nc.scalar.activation(
                    out=kq2T[sj][:], in_=ps_kq2[:],
                    func=mybir.ActivationFunctionType.Identity,
                    bias=bqk_ap, scale=1.0,
                )
            else:
                nc.vector.tensor_scalar_add(kqT[sj][:], ps_kq[:], bkq_ap)
                nc.vector.tensor_scalar_add(kq2T[sj][:], ps_kq2[:], bqk_ap)

        # qc0's ScalarE pairs interleave with the projections (exp-paced,
        # DMA-paced); its quad pairs (0,1 -> DVE) come last so the phase-A
        # proj ladder never waits on a 1.2us DVE op
        proj(0)
        proj(1)
        pair_block(0, 2)
        pair_block(0, 3)
        proj(2)
        pair_block(0, 4)
        pair_block(0, 5)
        proj(3)
        pair_block(0, 6)
        pair_block(0, 7)
        pair_block(0, 0)
        pair_block(0, 1)

        # ---- Phases B/C/D: qc 1..3 pair blocks with V units + ctx pops as
        # PE filler.  V units 6/7 sit in phase C behind the bf16 seq stream ----
        # quad pairs (0,1) spread out within each qc so gpsimd gets slack
        QCO = [0, 2, 3, 1, 4, 5, 6, 7]
        for i, p in enumerate(QCO):  # qc = 1
            pair_block(1, p)
            if i < 6:
                v_unit(i)
            if i == 1:
                emit_correction()
            if i >= 2:
                pop_ctx(2)
        for i, p in enumerate(QCO):  # qc = 2
            pair_block(2, p)
            if i < 2:
                v_unit(6 + i)
                pop_ctx(1)
            else:
                pop_ctx(2)
        for i, p in enumerate(QCO):  # qc = 3
            pair_block(3, p)
            pop_ctx(2)
        while pending_ctx:
            pop_ctx(len(pending_ctx), force=True)


_NC_CACHE = None


def _get_nc():
    global _NC_CACHE
    if _NC_CACHE is None:
        nc = bacc.Bacc("TRN2", target_bir_lowering=False, debug=False)
        _emit(nc)
        _NC_CACHE = nc
    return _NC_CACHE


def make_in_maps(seq, mask, Wq, bq, Wk, bk, Wv, bv):
    bf16 = ml_dtypes.bfloat16
    f16 = np.float16
    f8 = ml_dtypes.float8_e4m3
    seq = np.asarray(seq, dtype=np.float32)
    mask = np.asarray(mask).astype(bool)
    wkq = np.concatenate(
        [np.asarray(Wk, dtype=np.float32), np.asarray(Wq, dtype=np.float32)], axis=1
    )  # [D, 128]
    wqk = np.concatenate(
        [np.asarray(Wq, dtype=np.float32), np.asarray(Wk, dtype=np.float32)], axis=1
    )
    # DoubleRow layout [p, c, i, f] for row index d = 256c + 2p + i, contiguous
    wkq_h = np.ascontiguousarray(
        (wkq * 32.0).astype(f8).reshape(DR, 128, 2, 128).transpose(1, 0, 2, 3)
    )
    wqk_h = np.ascontiguousarray(
        (wqk * 32.0).astype(f8).reshape(DR, 128, 2, 128).transpose(1, 0, 2, 3)
    )
    wv_h = np.ascontiguousarray(
        np.asarray(Wv, dtype=np.float32).astype(bf16).reshape(DCH, 128, F).transpose(1, 0, 2)
    )
    cold = np.zeros((128, COLD_B), dtype=np.uint8)
    cold[:, 0:1024] = wv_h.reshape(128, 512).view(np.uint8)
    cold[:, 1024:1280] = np.eye(128, dtype=bf16).view(np.uint8)
    ih = np.zeros((128, 65), dtype=f16)
    ih[0:65] = np.eye(65, dtype=f16)
    cold[:, 1280:1410] = ih.view(np.uint8)
    hot = np.zeros((NCORES, 128, HOT_B), dtype=np.uint8)
    hot[:, :, 0:1024] = wkq_h.reshape(128, 1024).view(np.uint8)
    hot[:, :, 1024:2048] = wqk_h.reshape(128, 1024).view(np.uint8)
    in_maps = []
    for b in range(NCORES):
        seqT = np.ascontiguousarray(seq[b].T)  # [D, S] f32
        # fp8, partition-outermost: [p, sj, c, i, t]
        sf8 = np.ascontiguousarray(
            seqT.astype(f8).reshape(DR, 128, 2, NSJ, SC).transpose(1, 3, 0, 2, 4)
        )
        # bf16, partition-outermost: [p, sj, c, t]
        sb16 = np.ascontiguousarray(
            seqT.astype(bf16).reshape(DCH, 128, NSJ, SC).transpose(1, 2, 0, 3)
        )
        misc = np.zeros((128, 3 + KCH), dtype=np.float32)
        misc[0:F, 0] = 32.0 * np.asarray(bk, dtype=np.float32)
        misc[64:128, 0] = 32.0 * np.asarray(bq, dtype=np.float32)
        misc[0:F, 1] = 32.0 * np.asarray(bq, dtype=np.float32)
        misc[64:128, 1] = 32.0 * np.asarray(bk, dtype=np.float32)
        misc[0:F, 2] = np.asarray(bv, dtype=np.float32)
        # keep-mask: misc[p, 3+c] = 0.0 if key c*128+p is masked out else 1.0
        misc[:, 3:] = np.where(mask[b], np.float32(0.0), np.float32(1.0)).reshape(
            KCH, 128
        ).T
        hot[b, :, 2048:2124] = misc.view(np.uint8)
        in_maps.append(
            {
                "seqf8": sf8,
                "seqb": sb16,
                "ch": hot[b],
                "cc": cold,
            }
        )
    return in_maps


def run(in_maps, trace=False, **kw):
    nc = _get_nc()
    return run_bass_kernel_spmd(
        nc, in_maps, core_ids=list(range(NCORES)), trace=trace, **kw
    )


def kernel(seq, mask, Wq, bq, Wk, bk, Wv, bv):
    in_maps = make_in_maps(seq, mask, Wq, bq, Wk, bk, Wv, bv)
    res = run(in_maps)
    out = np.stack(
        [np.asarray(res.results[i]["out"], dtype=np.float32) for i in range(NCORES)],
        axis=0,
    )
    return out


# revision 25
# speedup vs baseline: 1.1218x; 1.1218x over previous
"""Single attention head (B=8, S=2048, D_IN=1024, D_OUT=64) on 8 TRN2 NeuronCores.

Strategy: pure data-parallel over batch -- core b computes batch element b's
full attention head. No collectives.

v9 -- the v3 baseline schedule (exp-paced, qc-major, deferred ctx pops)
with four independently-verified upgrades:
  - ~4.5us of junk matmuls on a memset tile (no DMA dependency) pull the
    PE HAM clock-gate through the SHORT window to 8/8 while the first
    input DMAs are still in flight (first-chunk DMA latency is ~5us);
    v3's warmups waited on the consts DMA, so all of phase A ran at the
    1.2GHz cold clock.
  - The consts DMA is split hot (wkq/wqk/misc -- everything phase A
    needs) / cold (wv/identities, first needed by the phase-B V units),
    and seqf8[0] lands as c0 / c1 / c2c3 chunks, so the first projection
    starts as early as the DMA pipe allows.  Host layouts are
    partition-outermost so any [sj, c-range] slice is one
    contiguous-per-partition descriptor.  The scalar HWDGE queue gets
    only early never-blocking issues (a credit-blocked issue instruction
    stalls the exp stream behind it in ScalarE's FIFO).
  - finalize runs in fp16 (PE transpose w/ fp16 identity): half the f32
    transpose cycles and 2x DVE rate on the psum drain.  |num| <~ 1e3,
    den ~ 1.7e3: well inside fp16 range, 4.9e-4 rel quantization.
  - The very last exp (qc3, pair 7) is split into two half-tiles and
    qc3's output DMA into two chunks, shortening the serial tail after
    the final exp by ~1us.
Per-core dataflow otherwise identical to v3 (see its docstring): fp8
DoubleRow K/Q projections with x32-scaled stacked weights, score pairs
co-run on disjoint PE row groups, one exp per [128,1024] pair tile, mask
applied via zeroed V rows + keep-column denominator, ctx accumulated as
ctxT[65, q] with deferred pops as PE filler.
"""

import numpy as np
import ml_dtypes

import concourse.bass as bass  # noqa: F401
import concourse.mybir as mybir
import concourse.tile as tile
from concourse import bacc
from concourse.bass_utils import run_bass_kernel_spmd

B, S, D, F = 8, 2048, 1024, 64
NCORES = 8
BF = mybir.dt.bfloat16
F16 = mybir.dt.float16
F8 = mybir.dt.float8e4
F32 = mybir.dt.float32
SCALE = 1.0 / (1024.0 * float(np.sqrt(np.float32(S))))
SC = 512
NSJ = S // SC
KCH = S // 128
DCH = D // 128
DR = D // 256
HOT_B = 2124  # wkq 1024 | wqk 1024 | misc 76
COLD_B = 1410  # wv 1024 | identb 256 | identh 130


def _emit(nc):
    seqf8_d = nc.declare_dram_parameter("seqf8", [128, NSJ, DR, 2, SC], F8, isOutput=False)
    seqb_d = nc.declare_dram_parameter("seqb", [128, NSJ, DCH, SC], BF, isOutput=False)
    ch_d = nc.declare_dram_parameter("ch", [128, HOT_B], mybir.dt.uint8, isOutput=False)
    cc_d = nc.declare_dram_parameter("cc", [128, COLD_B], mybir.dt.uint8, isOutput=False)
    out_d = nc.declare_dram_parameter("out", [S, F], F32, isOutput=True)

    with tile.TileContext(nc) as tc:
        _body(nc, tc, seqf8_d, seqb_d, ch_d, cc_d, out_d)
    nc.compile()


def _body(nc, tc, seqf8_d, seqb_d, ch_d, cc_d, out_d):
    from contextlib import ExitStack

    with ExitStack() as ctx:
        const = ctx.enter_context(tc.tile_pool(name="const", bufs=1))
        big = ctx.enter_context(tc.tile_pool(name="big", bufs=1))
        sbw = ctx.enter_context(tc.tile_pool(name="sbw", bufs=1))
        ps = ctx.enter_context(tc.tile_pool(name="ps", space="PSUM", bufs=1))

        # ---- HAM warmup + exp-table preload: no DMA dependencies ----
        warm_sb = const.tile([128, SC], F8, name="warm_sb")
        nc.gpsimd.memset(warm_sb[:], 0)
        dummy_sb = const.tile([1, 1], F32, name="dummy_sb")
        nc.scalar.activation(
            out=dummy_sb[:],
            in_=warm_sb[0:1, 0:4].bitcast(F32),
            func=mybir.ActivationFunctionType.Exp,
            scale=1.0,
        )
        for i in range(13):
            ps_warm = ps.tile([128, SC], F32, tag="pk", bufs=2, name=f"ps_warm{i}")
            nc.tensor.matmul(
                ps_warm[:], warm_sb[:, 0:128], warm_sb[:], start=True, stop=True
            )

        # ---- input DMAs: scalar queue only gets early never-blocking
        # issues; the bulk streams on sync, ordered by first use ----
        hot_sb = const.tile([128, HOT_B], mybir.dt.uint8, name="hot_sb")
        cold_sb = const.tile([128, COLD_B], mybir.dt.uint8, name="cold_sb")
        f8_00 = big.tile([128, 1, 2, SC], F8, name="f8_00")
        f8_01 = big.tile([128, 1, 2, SC], F8, name="f8_01")
        f8_0b = big.tile([128, 2, 2, SC], F8, name="f8_0b")
        seqf8 = [None] + [
            big.tile([128, DR, 2, SC], F8, name=f"seqf8_{j}") for j in range(1, NSJ)
        ]
        seqbh = [
            big.tile([128, DCH // 2, SC], BF, name=f"seqb_{j}_{h}")
            for j in range(2)
            for h in range(2)
        ]
        seqbf = [big.tile([128, DCH, SC], BF, name=f"seqb_{j}") for j in (2, 3)]

        nc.scalar.dma_start(out=f8_00[:], in_=seqf8_d[:, 0, 0:1, :, :])
        nc.sync.dma_start(out=hot_sb[:], in_=ch_d.ap())
        nc.scalar.dma_start(out=f8_01[:], in_=seqf8_d[:, 0, 1:2, :, :])
        nc.sync.dma_start(out=f8_0b[:], in_=seqf8_d[:, 0, 2:4, :, :])
        nc.scalar.dma_start(out=seqf8[1][:], in_=seqf8_d[:, 1])
        nc.sync.dma_start(out=seqf8[2][:], in_=seqf8_d[:, 2])
        nc.sync.dma_start(out=seqf8[3][:], in_=seqf8_d[:, 3])
        nc.scalar.dma_start(out=cold_sb[:], in_=cc_d.ap())
        nc.scalar.dma_start(out=seqbh[0][:], in_=seqb_d[:, 0, 0:4, :])
        nc.sync.dma_start(out=seqbh[1][:], in_=seqb_d[:, 0, 4:8, :])
        nc.scalar.dma_start(out=seqbh[2][:], in_=seqb_d[:, 1, 0:4, :])
        nc.sync.dma_start(out=seqbh[3][:], in_=seqb_d[:, 1, 4:8, :])
        nc.sync.dma_start(out=seqbf[0][:], in_=seqb_d[:, 2])
        nc.sync.dma_start(out=seqbf[1][:], in_=seqb_d[:, 3])

        def seqb_half(u):
            sj, h = divmod(u, 2)
            if sj < 2:
                return seqbh[u][:, :, :]
            return seqbf[sj - 2][:, 4 * h : 4 * h + 4, :]

        wkq_sb = hot_sb[:, 0:1024].bitcast(F8).rearrange(
            "p (c i f) -> p c i f", c=DR, i=2
        )
        wqk_sb = hot_sb[:, 1024:2048].bitcast(F8).rearrange(
            "p (c i f) -> p c i f", c=DR, i=2
        )
        misc_sb = hot_sb[:, 2048:2124].bitcast(F32)
        wv_sb = cold_sb[:, 0:1024].bitcast(BF).rearrange("p (c f) -> p c f", c=DCH)
        identb_sb = cold_sb[:, 1024:1280].bitcast(BF)
        identh_sb = cold_sb[:, 1280:1410].bitcast(F16)

        kqT = [big.tile([128, SC], BF, name=f"kqT_{j}") for j in range(NSJ)]
        kq2T = [big.tile([128, SC], BF, name=f"kq2T_{j}") for j in range(NSJ)]
        vT = [big.tile([F, SC], BF, name=f"vT_{j}") for j in range(NSJ)]
        v_sbs = [big.tile([128, 4, F + 1], BF, name=f"v_sb{j}") for j in range(NSJ)]
        out_sbs = [big.tile([128, 4, F], F32, name=f"out_sb{q}") for q in range(4)]
        out_r = out_d.ap().rearrange("(c p) f -> p c f", p=128)

        bkq_ap = misc_sb[:, 0:1]
        bqk_ap = misc_sb[:, 1:2]
        bv_ap = misc_sb[0:F, 2:3]
        mask01 = misc_sb[:, 3:]

        for j in range(NSJ):
            nc.gpsimd.tensor_copy(v_sbs[j][:, :, F], mask01[:, 4 * j : 4 * j + 4])

        ctx_tiles = {}
        pending_ctx = []

        def emit_ctx(qc, p, wA, wB):
            ctx_ps = ctx_tiles[qc]
            ka, kb = 2 * p, 2 * p + 1
            nc.tensor.matmul(
                ctx_ps[:],
                v_sbs[ka // 4][:, ka % 4, :],
                wA,
                start=(p == 0),
                stop=False,
            )
            nc.tensor.matmul(
                ctx_ps[:],
                v_sbs[kb // 4][:, kb % 4, :],
                wB,
                start=False,
                stop=(p == KCH // 2 - 1),
            )

        def pop_ctx(n):
            for _ in range(min(n, len(pending_ctx))):
                qc, p, wA, wB = pending_ctx.pop(0)
                emit_ctx(qc, p, wA, wB)
                if p == KCH // 2 - 1:
                    finalize(qc)

        def pair_block(qc, p):
            if qc not in ctx_tiles:
                ctx_tiles[qc] = ps.tile(
                    [F + 1, SC], F32, tag="ctx", bufs=2, name=f"ctx_ps{qc}"
                )
            ka, kb = 2 * p, 2 * p + 1
            ps_pair = ps.tile(
                [128, 2 * SC], F32, tag="pair", bufs=2, name=f"ps_pair_{qc}_{p}"
            )
            nc.tensor.matmul(
                ps_pair[:, 0:SC],
                kqT[ka // 4][0:F, (ka % 4) * 128 : (ka % 4 + 1) * 128],
                kq2T[qc][0:F, :],
                start=True,
                stop=True,
            )
            nc.tensor.matmul(
                ps_pair[:, SC : 2 * SC],
                kq2T[kb // 4][64:128, (kb % 4) * 128 : (kb % 4 + 1) * 128],
                kqT[qc][64:128, :],
                start=True,
                stop=True,
            )
            if qc == 3 and p == 7:
                # the very last weights: two half-exps so the final ctx
                # matmuls start ~0.7us earlier (shorter serial tail)
                eA = sbw.tile([128, SC], BF, tag="expq", bufs=16, name="expq_37a")
                eB = sbw.tile([128, SC], BF, tag="expq", bufs=16, name="expq_37b")
                for h, e in enumerate((eA, eB)):
                    nc.scalar.activation(
                        out=e[:],
                        in_=ps_pair[:, h * SC : (h + 1) * SC],
                        func=mybir.ActivationFunctionType.Exp,
                        scale=SCALE,
                    )
                pending_ctx.append((qc, p, eA[:], eB[:]))
            else:
                expq = sbw.tile(
                    [128, 2 * SC], BF, tag="expq", bufs=16, name=f"expq_{qc}_{p}"
                )
                nc.scalar.activation(
                    out=expq[:],
                    in_=ps_pair[:],
                    func=mybir.ActivationFunctionType.Exp,
                    scale=SCALE,
                )
                pending_ctx.append((qc, p, expq[:, 0:SC], expq[:, SC : 2 * SC]))

        vps = {}

        def v_unit(u):
            sj, h = divmod(u, 2)
            if h == 0:
                vps[sj] = ps.tile([F, SC], F32, tag="pk", bufs=2, name=f"ps_v{sj}")
            for c in range(4 * h, 4 * h + 4):
                nc.tensor.matmul(
                    vps[sj][:],
                    wv_sb[:, c, :],
                    seqb_half(u)[:, c - 4 * h, :],
                    start=(c == 0),
                    stop=(c == DCH - 1),
                )
            if h == 1:
                nc.vector.tensor_scalar_add(vT[sj][:], vps[sj][:], bv_ap)
                for i in range(4):
                    t = 4 * sj + i
                    vtp = ps.tile([128, F], BF, tag="pk", bufs=2, name=f"vtp{t}")
                    nc.tensor.transpose(
                        vtp[:],
                        vT[sj][:, i * 128 : (i + 1) * 128],
                        identb_sb[0:F, 0:F],
                    )
                    nc.vector.tensor_scalar_mul(
                        v_sbs[sj][:, i, 0:F], vtp[:], mask01[:, t : t + 1]
                    )

        def finalize(qc):
            ctx_ps = ctx_tiles.pop(qc)
            ctxTq = sbw.tile([F + 1, SC], F16, tag="ctxTq", bufs=2, name=f"ctxTq{qc}")
            nc.vector.tensor_copy(ctxTq[:], ctx_ps[:])
            for i in range(SC // 128):
                t = qc * 4 + i
                ctp = ps.tile([128, F + 1], F16, tag="pk", bufs=2, name=f"ctp{t}")
                nc.tensor.transpose(
                    ctp[:],
                    ctxTq[:, i * 128 : (i + 1) * 128],
                    identh_sb[0 : F + 1, 0 : F + 1],
                )
                rec = sbw.tile([128, 1], F32, tag="rec", bufs=4, name=f"rec{t}")
                nc.vector.reciprocal(rec[:], ctp[:, F : F + 1])
                nc.vector.tensor_scalar_mul(
                    out_sbs[qc][:, i, :], ctp[:, 0:F], rec[:]
                )
                if qc == 3 and i == 1:
                    nc.sync.dma_start(
                        out=out_r[:, 12:14, :], in_=out_sbs[3][:, 0:2, :]
                    )
            if qc == 3:
                nc.sync.dma_start(out=out_r[:, 14:16, :], in_=out_sbs[3][:, 2:4, :])
            else:
                nc.sync.dma_start(
                    out=out_r[:, qc * 4 : (qc + 1) * 4, :],
                    in_=out_sbs[qc][:],
                )

        # ---- Phase A: K/Q projections with q-chunk 0's pair blocks ----
        for sj in range(NSJ):
            ps_kq = ps.tile([128, SC], F32, tag="pk", bufs=2, name=f"ps_kq{sj}")
            ps_kq2 = ps.tile([128, SC], F32, tag="pk", bufs=2, name=f"ps_kq2_{sj}")
            for c in range(DR):
                if sj == 0:
                    if c < 2:
                        rhs = (f8_00 if c == 0 else f8_01)[:, 0, :, :]
                    else:
                        rhs = f8_0b[:, c - 2, :, :]
                else:
                    rhs = seqf8[sj][:, c, :, :]
                st = dict(start=(c == 0), stop=(c == DR - 1))
                nc.tensor.matmul(
                    ps_kq[:], wkq_sb[:, c, :, :], rhs,
                    perf_mode=mybir.MatmulPerfMode.DoubleRow, **st
                )
                nc.tensor.matmul(
                    ps_kq2[:], wqk_sb[:, c, :, :], rhs,
                    perf_mode=mybir.MatmulPerfMode.DoubleRow, **st
                )
            nc.vector.tensor_scalar_add(kqT[sj][:], ps_kq[:], bkq_ap)
            nc.vector.tensor_scalar_add(kq2T[sj][:], ps_kq2[:], bqk_ap)
            pair_block(0, 2 * sj)
            pair_block(0, 2 * sj + 1)

        # ---- Phases B/C/D: qc 1..3 pair blocks, exp-paced ----
        for p in range(KCH // 2):  # qc = 1
            pair_block(1, p)
            if p < 6:
                v_unit(p)
            else:
                pop_ctx(2)
        for p in range(KCH // 2):  # qc = 2
            pair_block(2, p)
            if p < 2:
                v_unit(6 + p)
                pop_ctx(1)
            else:
                pop_ctx(2)
        for p in range(KCH // 2):  # qc = 3
            pair_block(3, p)
            pop_ctx(2)
        pop_ctx(len(pending_ctx))
        pop_ctx(len(pending_ctx))


_NC_CACHE = None


def _get_nc():
    global _NC_CACHE
    if _NC_CACHE is None:
        nc = bacc.Bacc("TRN2", target_bir_lowering=False, debug=False)
        _emit(nc)
        _NC_CACHE = nc
    return _NC_CACHE


def make_in_maps(seq, mask, Wq, bq, Wk, bk, Wv, bv):
    bf16 = ml_dtypes.bfloat16
    f16 = np.float16
    f8 = ml_dtypes.float8_e4m3
    seq = np.asarray(seq, dtype=np.float32)
    mask = np.asarray(mask).astype(bool)
    wkq = np.concatenate(
        [np.asarray(Wk, dtype=np.float32), np.asarray(Wq, dtype=np.float32)], axis=1
    )
    wqk = np.concatenate(
        [np.asarray(Wq, dtype=np.float32), np.asarray(Wk, dtype=np.float32)], axis=1
    )
    wkq_h = np.ascontiguousarray(
        (wkq * 32.0).astype(f8).reshape(DR, 128, 2, 128).transpose(1, 0, 2, 3)
    )
    wqk_h = np.ascontiguousarray(
        (wqk * 32.0).astype(f8).reshape(DR, 128, 2, 128).transpose(1, 0, 2, 3)
    )
    wv_h = np.ascontiguousarray(
        np.asarray(Wv, dtype=np.float32).astype(bf16).reshape(DCH, 128, F).transpose(1, 0, 2)
    )
    cold = np.zeros((128, COLD_B), dtype=np.uint8)
    cold[:, 0:1024] = wv_h.reshape(128, 512).view(np.uint8)
    cold[:, 1024:1280] = np.eye(128, dtype=bf16).view(np.uint8)
    ih = np.zeros((128, 65), dtype=f16)
    ih[0:65] = np.eye(65, dtype=f16)
    cold[:, 1280:1410] = ih.view(np.uint8)
    hot = np.zeros((NCORES, 128, HOT_B), dtype=np.uint8)
    hot[:, :, 0:1024] = wkq_h.reshape(128, 1024).view(np.uint8)
    hot[:, :, 1024:2048] = wqk_h.reshape(128, 1024).view(np.uint8)
    in_maps = []
    for b in range(NCORES):
        seqT = np.ascontiguousarray(seq[b].T)
        # fp8, partition-outermost: [p, sj, c, i, t]
        sf8 = np.ascontiguousarray(
            seqT.astype(f8).reshape(DR, 128, 2, NSJ, SC).transpose(1, 3, 0, 2, 4)
        )
        # bf16, partition-outermost: [p, sj, c, t]
        sb16 = np.ascontiguousarray(
            seqT.astype(bf16).reshape(DCH, 128, NSJ, SC).transpose(1, 2, 0, 3)
        )
        misc = np.zeros((128, 3 + KCH), dtype=np.float32)
        misc[0:F, 0] = 32.0 * np.asarray(bk, dtype=np.float32)
        misc[64:128, 0] = 32.0 * np.asarray(bq, dtype=np.float32)
        misc[0:F, 1] = 32.0 * np.asarray(bq, dtype=np.float32)
        misc[64:128, 1] = 32.0 * np.asarray(bk, dtype=np.float32)
        misc[0:F, 2] = np.asarray(bv, dtype=np.float32)
        misc[:, 3:] = np.where(mask[b], np.float32(0.0), np.float32(1.0)).reshape(
            KCH, 128
        ).T
        hot[b, :, 2048:2124] = misc.view(np.uint8)
        in_maps.append(
            {
                "seqf8": sf8,
                "seqb": sb16,
                "ch": hot[b],
                "cc": cold,
            }
        )
    return in_maps


def run(in_maps, trace=False, **kw):
    nc = _get_nc()
    return run_bass_kernel_spmd(
        nc, in_maps, core_ids=list(range(NCORES)), trace=trace, **kw
    )


def kernel(seq, mask, Wq, bq, Wk, bk, Wv, bv):
    in_maps = make_in_maps(seq, mask, Wq, bq, Wk, bk, Wv, bv)
    res = run(in_maps)
    out = np.stack(
        [np.asarray(res.results[i]["out"], dtype=np.float32) for i in range(NCORES)],
        axis=0,
    )
    return out
